# revision 58
# baseline (speedup 1.0000x reference)
"""Trainium2 Bass kernel for nn_DiscreteTimeNeuralGraph.

Strategy (8 NeuronCores, batch-parallel):
  - Shard the batch of 32 across 8 cores (4 samples each). All weights
    replicated; weight DMA split in three (downsample chunk first) so the
    downsample path starts before the bulky main-loop weights land.
  - Downsample path on-device; BatchNorm batch stats via per-core partial
    moments + one small AllGather each (collective AllReduce has a ~2x
    higher fixed cost in practice) followed by a local free-dim reduce.
  - All zero-padding of activation borders is done with strided memsets on
    the Vector engine, never with DMAs (which would serialize on a queue).
  - 8 graph layers: depthwise 3x3 conv as 9 rect-clipped diagonal matmuls
    on PE accumulating in PSUM; channel mix (pruned 512x512 weight, dense)
    as blocked fp16 matmuls; instance-norm stats on VectorE (bn_stats on
    PSUM); instnorm+ReLU fused into one ScalarE activation reading PSUM
    and writing the next layer's activations (f32r).
  - Engine balance: for layers 3..8, the depthwise conv of channel group 2
    runs entirely on the Vector engine (per-channel scalar_tensor_tensor
    accumulation chains, emitted one layer ahead right after the producing
    activation) while PE keeps groups 0/1/3 and the channel mix. Starting
    at L3 (not L2) avoids stalling on chains that would have to hide under
    the short first layer.
  - Precision: activations X and PE depthwise path in f32r; the depthwise
    outputs D and the mix weights are fp16 (10-bit mantissa), which
    halves their SBUF footprint; PSUM accumulation stays f32.
  - Readout: center 2x2 mean (folded into fc weights, accumulated
    per-sample as L8 finishes) + fc matmul.

Top-k threshold for the pruned graph weight is computed on host
(np.partition) -- it is weight preprocessing of a replicated input.
"""
import numpy as np

import concourse.bass as bass
import concourse.tile as tile
from concourse import bacc, mybir
from concourse.bass_utils import run_bass_kernel_spmd

F32 = mybir.dt.float32
F32R = mybir.dt.float32r
AF = mybir.ActivationFunctionType

N_CORES = 8
B = 32
BPC = B // N_CORES          # 4 samples per core
DIM = 512
DS = 128
FEAT = 256
LAYERS = 8
IMG = 128
OUT = 1000
EPS = 1e-5
HALF = IMG // 4 // 2 - 1    # 15
PRUNE = 0.9

# mega-weight column layout (f32r, [128, WCOLS]); ds-phase block first so
# the first DMA chunk unblocks conv1 quickly.
W1X_OFF = 0                  # 3 dx-taps x [128,128] for conv1
W2D_OFF = W1X_OFF + 3 * 128  # 9 taps x [128,128] diag-dup for conv2
W3_OFF = W2D_OFF + 9 * 128   # [128,128] conv3 (w3 stacked twice on K)
BN1_OFF = W3_OFF + 128       # [128, 2] bn1 gamma/beta (dup across halves)
BN2_OFF = BN1_OFF + 2        # [128, 2]
DS_COLS = BN2_OFF + 2        # end of ds-phase chunk
WDW_OFF = DS_COLS              # 4 groups x 9 taps x [128,128] diag, f32r
FCW_OFF = WDW_OFF + 36 * 128   # 2 kg x [128, 1000] fc lhsT (x0.25 pooled)
FCB_OFF = FCW_OFF + 2 * 1000   # [128, 8] fc bias chunks
NGB_OFF = FCB_OFF + 8          # [128, 8] instnorm gamma/beta per group
WDWS_OFF = NGB_OFF + 8         # [128, 36] compact dw taps (per-channel) for
                               # the DVE-offloaded depthwise instances
WSB_COLS = WDWS_OFF + 36       # end of the f32r weight tile
# fp16 mix weights (packed in pairs into the f32 dram buffer):
# 4 kg x [128,512] blocks; D is fp16 so the mix runs fp16 x fp16
WHX_OFF = WSB_COLS             # in f32-column units of the dram buffer
WHX_HCOLS = 4 * 512
WCOLS = WHX_OFF + WHX_HCOLS // 2

XP_BUFS = 20
XW = 34                        # X row width: [z | 32 interior | z] (f32r)
BF16 = mybir.dt.float16  # fp16: 10-bit mantissa, same PE/DVE speed as bf16

# tap order: full-coverage tap first (start=True zeroes the psum region)
TAPS = [(1, 1), (0, 0), (0, 1), (0, 2), (1, 0), (1, 2), (2, 0), (2, 1), (2, 2)]


def _clip(lo, hi, lo2, hi2):
    return max(lo, lo2), min(hi, hi2)


def di_col(dx):
    # column index of conv1 tap dx within w1x block (emission order 1,0,2)
    return {1: 0, 0: 1, 2: 2}[dx]


def build_nc():
    nc = bacc.Bacc(num_devices=N_CORES)
    x4 = nc.dram_tensor("x4", [BPC, 3, IMG, IMG], F32R, kind="ExternalInput").ap()
    wts = nc.dram_tensor("wts", [128, WCOLS], F32R, kind="ExternalInput").ap()
    y4 = nc.dram_tensor("y4", [BPC, OUT], F32, kind="ExternalOutput").ap()

    with tile.TileContext(nc) as tc:
        with (
            tc.tile_pool(name="wp", bufs=1) as wp,
            tc.tile_pool(name="small", bufs=1) as small,
            tc.tile_pool(name="psA", bufs=2, space="PSUM") as psA,
            tc.tile_pool(name="psB", bufs=3, space="PSUM") as psB,
            tc.tile_pool(name="dram", bufs=1, space="DRAM") as dram,
        ):
            w_sb = wp.tile([128, WSB_COLS], F32R)
            w32 = w_sb.bitcast(F32)
            whx_sb = wp.tile([128, WHX_HCOLS], BF16)

            def wcols(off, n):
                return w_sb[:, off:off + n]

            def wdw_blk(g, t):
                return wcols(WDW_OFF + (g * 9 + t) * 128, 128)

            def wmix_blk(kg, mg):
                c = kg * 512 + mg * 128
                return whx_sb[:, c:c + 128]

            eps_t = small.tile([128, 1], F32)
            nc.vector.memset(eps_t, EPS)

            # ---------------- downsample ----------------
            with tc.tile_pool(name="ds1", bufs=1) as ds1:
                # im2col9: partition p = 32*s + 3*dy + c ; free = (oy 64, ix' 130)
                # ix' = ix + 1 (x padded by 1 on both sides)
                im9 = ds1.tile([128, 64 * 130], F32R)
                im9r = im9.rearrange("p (y x) -> p y x", y=64, x=130)
                # zero the x pads (cols 0 and 129) with strided memsets
                for xc in (0, 129):
                    im9_pads = bass.AP(tensor=im9.tensor,
                                       offset=im9.offset + xc,
                                       ap=[im9.ap[0], [130, 64]])
                    nc.vector.memset(im9_pads.bitcast(F32), 0.0)
                # row oy=0 zeroed (only the dy=0 partitions keep it)
                nc.vector.memset(im9r[:, 0:1, :].bitcast(F32), 0.0)
                # x rows: iy = 2*oy + dy - 1
                # partition base: sample s -> 64*(s%2) + 27*(s//2)
                x4r = x4.rearrange("s c (y2 two) x -> s c y2 two x", two=2)
                for s in range(BPC):
                    for dy in range(3):
                        p0 = 64 * (s % 2) + 27 * (s // 2) + 3 * dy
                        if dy == 0:
                            # oy in [1,64): iy = 2*(oy-1)+1
                            nc.sync.dma_start(
                                out=im9r[p0:p0 + 3, 1:64, 1:129],
                                in_=x4r[s, :, 0:63, 1, :])
                        elif dy == 1:
                            nc.sync.dma_start(
                                out=im9r[p0:p0 + 3, :, 1:129],
                                in_=x4r[s, :, :, 0, :])
                        else:
                            nc.sync.dma_start(
                                out=im9r[p0:p0 + 3, :, 1:129],
                                in_=x4r[s, :, :, 1, :])
                # weight DMAs after the input: ds-phase chunk unblocks conv1;
                # the bulky main-loop chunks overlap conv1/BN syncs. The
                # bf16-packed dw-tap region must move as bf16 (an f32r-typed
                # DMA rounds mantissas, corrupting packed bf16 pairs).
                nc.sync.dma_start(out=w_sb[:, 0:DS_COLS], in_=wts[:, 0:DS_COLS])
                nc.sync.dma_start(out=w_sb[:, DS_COLS:WSB_COLS],
                                  in_=wts[:, DS_COLS:WSB_COLS])
                nc.sync.dma_start(out=whx_sb,
                                  in_=wts[:, WHX_OFF:WCOLS].bitcast(BF16))

                # conv1: out h1 [128 = 64*(s//2)+ch, (s%2, oy 64, ox 64)]
                h1 = ds1.tile([128, 8192], F32)
                h1r = h1.rearrange("p (sh y x) -> p sh y x", sh=2, y=64, x=64)
                # im2col x-read: ix' = 2*ox + dx (x2 = ox + dx//2, tx = dx%2)
                # paired matmul: K=54 block-diag covers samples (q, q+2):
                # out partitions 0-63 <- sample q, 64-127 <- sample q+2.
                im9x = im9.rearrange("p (y x2 two) -> p y x2 two", x2=65, two=2)
                for q in range(2):
                    for yb in range(4):           # 16-oy blocks
                        for h in range(2):
                            pc1 = psA.tile([128, 512], F32, tag="a",
                                           name="pc1")
                            pc1r = pc1.rearrange("p (y x) -> p y x", y=8, x=64)
                            oy0 = yb * 16 + h * 8
                            for di, dx in enumerate([1, 0, 2]):
                                rhs = im9x[64 * q:64 * q + 54, oy0:oy0 + 8,
                                           dx // 2:dx // 2 + 64, dx % 2]
                                lhsT = w_sb[64 * q:64 * q + 54,
                                            W1X_OFF + di_col(dx) * 128:
                                            W1X_OFF + di_col(dx) * 128 + 128]
                                nc.tensor.matmul(pc1r, lhsT, rhs,
                                                 start=(di == 0), stop=(di == 2),
                                                 tile_position=(64 * q, 0))
                            if (q + yb + h) % 2 == 0:
                                nc.scalar.copy(
                                    out=h1r[:, q, oy0:oy0 + 8, :], in_=pc1)
                            else:
                                nc.vector.tensor_copy(
                                    out=h1r[:, q, oy0:oy0 + 8, :], in_=pc1)

                # BN1 partial stats
                st1 = small.tile([128, 16, 6], F32)
                for i in range(16):
                    nc.vector.bn_stats(out=st1[:, i, :],
                                       in_=h1[:, i * 512:(i + 1) * 512])
                mv1 = small.tile([128, 2], F32)
                nc.vector.bn_aggr(out=mv1, in_=st1)
                # prescaled moments: global mean = sum of the 16 (core, half)
                # local means / 16, ditto E[x^2]
                sums1 = small.tile([128, 2], F32)
                tmp1 = small.tile([128, 1], F32)
                nc.vector.tensor_scalar_mul(out=sums1[:, 0:1], in0=mv1[:, 0:1],
                                            scalar1=1.0 / 16.0)
                nc.vector.tensor_mul(out=tmp1, in0=mv1[:, 0:1], in1=mv1[:, 0:1])
                nc.vector.tensor_add(out=tmp1, in0=tmp1, in1=mv1[:, 1:2])
                nc.vector.tensor_scalar_mul(out=sums1[:, 1:2], in0=tmp1,
                                            scalar1=1.0 / 16.0)
                bn1_in = dram.tile([128, 2], F32)
                bn1_g8 = dram.tile([8, 128, 2], F32)
                nc.gpsimd.dma_start(out=bn1_in, in_=sums1)
                nc.gpsimd.collective_compute(
                    "AllGather", mybir.AluOpType.bypass,
                    replica_groups=[list(range(N_CORES))],
                    ins=[bn1_in.opt()], outs=[bn1_g8.opt()])
                # both sample-halves of channel c land on partitions c and
                # c+64 via two parallel DMAs reading the same [ch, s, g, half]
                # pattern; one reduce over the 16 (g, half) entries.
                gath1 = small.tile([128, 2, 16], F32)
                src1 = bass.AP(tensor=bn1_g8.tensor, offset=bn1_g8.offset,
                               ap=[[2, 64], [1, 2], [256, 8], [128, 2]])
                nc.gpsimd.dma_start(out=gath1[0:64], in_=src1)
                nc.sync.dma_start(out=gath1[64:128], in_=src1)
                red1 = small.tile([128, 2], F32)
                nc.vector.tensor_reduce(out=red1, in_=gath1,
                                        axis=mybir.AxisListType.X,
                                        op=mybir.AluOpType.add)
                s1t1 = small.tile([128, 2], F32)
                _bn_scale_bias(nc, s1t1, red1, w32, BN1_OFF, eps_t, small)

                # apply BN1 + relu -> h1n (f32r), x padded to 66 (ix' = ix+1)
                h1n = ds1.tile([128, 2 * 64 * 66], F32R)
                h1nr3 = h1n.rearrange("p (sh y x) -> p sh y x",
                                      sh=2, y=64, x=66)
                for sh in range(2):
                    h1n_pads = bass.AP(tensor=h1n.tensor,
                                       offset=h1n.offset + 4224 * sh,
                                       ap=[h1n.ap[0], [66, 64], [65, 2]])
                    nc.vector.memset(h1n_pads.bitcast(F32), 0.0)
                # chunked apply for pipelining into conv2
                for sh in range(2):
                    for yh in range(2):
                        nc.scalar.activation(
                            out=h1nr3[:, sh, yh * 32:yh * 32 + 32, 1:65],
                            in_=h1r[:, sh, yh * 32:yh * 32 + 32, :],
                            func=AF.Relu,
                            scale=s1t1[:, 0:1], bias=s1t1[:, 1:2])

                # conv2: depthwise 3x3 stride 2 -> d2 [128, (sh, 32, 32)]
                # row iy = 2*oy + dy - 1 (unpadded), col ix' = 2*ox + dx (padded)
                h1nr = h1n.rearrange(
                    "p (sh y2 ty x2 tx) -> p sh y2 ty x2 tx",
                    sh=2, y2=32, ty=2, x2=33, tx=2)
                d2 = ds1.tile([128, 2048], F32R)
                for sh in range(2):
                    pd2 = psB.tile([128, 1024], F32, tag="b", name="pd2")
                    pd2r = pd2.rearrange("p (h y x) -> p h y x", h=2, y=16, x=32)
                    for h in range(2):
                        for ti, (dy, dx) in enumerate(TAPS):
                            oy0, oy1 = _clip(h * 16, h * 16 + 16,
                                             1 if dy == 0 else 0, 32)
                            if dy == 1:
                                ys, par = oy0, 0
                            elif dy == 0:
                                ys, par = oy0 - 1, 1
                            else:
                                ys, par = oy0, 1
                            rhs = h1nr[:, sh, ys:ys + (oy1 - oy0), par,
                                       dx // 2:dx // 2 + 32, dx % 2]
                            outp = pd2r[:, h, oy0 - h * 16:oy1 - h * 16, :]
                            t = TAPS.index((dy, dx))
                            nc.tensor.matmul(
                                outp, wcols(W2D_OFF + t * 128, 128), rhs,
                                start=(ti == 0), stop=(ti == len(TAPS) - 1))
                    nc.scalar.copy(out=d2[:, sh * 1024:(sh + 1) * 1024], in_=pd2)

                # conv3: 1x1, 64 -> 128 ; h3 [128=outc, (s, 1024px)]
                # BN2 partial stats interleave with the conv3 psum copies
                h3 = small.tile([128, 4096], F32)
                st2 = small.tile([128, 8, 6], F32)
                for a in range(2):
                    for nb in range(4):
                        pc3 = psA.tile([128, 512], F32, tag="a",
                                       name=f"pc3_{a}_{nb}")
                        nc.tensor.matmul(
                            pc3,
                            w_sb[64 * a:64 * a + 64, W3_OFF:W3_OFF + 128],
                            d2[64 * a:64 * a + 64, nb * 512:(nb + 1) * 512],
                            start=True, stop=True)
                        s_full = 2 * a + nb // 2
                        ci = s_full * 2 + (nb % 2)
                        dst = h3[:, ci * 512:ci * 512 + 512]
                        nc.scalar.copy(out=dst, in_=pc3)
                        nc.vector.bn_stats(out=st2[:, ci, :], in_=dst)
                mv2 = small.tile([128, 2], F32)
                nc.vector.bn_aggr(out=mv2, in_=st2)
                sums2 = small.tile([128, 2], F32)
                tmp2 = small.tile([128, 1], F32)
                nc.vector.tensor_scalar_mul(out=sums2[:, 0:1], in0=mv2[:, 0:1],
                                            scalar1=1.0 / 8.0)
                nc.vector.tensor_mul(out=tmp2, in0=mv2[:, 0:1], in1=mv2[:, 0:1])
                nc.vector.tensor_add(out=tmp2, in0=tmp2, in1=mv2[:, 1:2])
                nc.vector.tensor_scalar_mul(out=sums2[:, 1:2], in0=tmp2,
                                            scalar1=1.0 / 8.0)
                bn2_in = dram.tile([128, 2], F32)
                bn2_g8 = dram.tile([8, 128, 2], F32)
                nc.gpsimd.dma_start(out=bn2_in, in_=sums2)
                nc.gpsimd.collective_compute(
                    "AllGather", mybir.AluOpType.bypass,
                    replica_groups=[list(range(N_CORES))],
                    ins=[bn2_in.opt()], outs=[bn2_g8.opt()])
                gath2 = small.tile([128, 2, 8], F32)
                src2 = bass.AP(tensor=bn2_g8.tensor, offset=bn2_g8.offset,
                               ap=[[2, 128], [1, 2], [256, 8]])
                nc.gpsimd.dma_start(out=gath2, in_=src2)
                red2 = small.tile([128, 2], F32)
                nc.vector.tensor_reduce(out=red2, in_=gath2,
                                        axis=mybir.AxisListType.X,
                                        op=mybir.AluOpType.add)
                s2t2 = small.tile([128, 2], F32)
                _bn_scale_bias(nc, s2t2, red2, w32, BN2_OFF, eps_t, small)

            # ---------------- main loop ----------------
            with (
                tc.tile_pool(name="xp", bufs=XP_BUFS) as xp,
                tc.tile_pool(name="dp", bufs=16) as dp,
                tc.tile_pool(name="tp", bufs=4) as tp,
                tc.tile_pool(name="stp", bufs=4) as stp,
            ):
                def new_x_tile(name):
                    # pad columns (0, 33) of every xp slot were zeroed once
                    # below; applies only write the interior, so they persist.
                    return xp.tile([128, 32 * XW], F32R, tag="X", name=name)

                # one-time zeroing of the pad columns of all X slots: the
                # dummies are simultaneously live (kept alive by the reads
                # below), so by pigeonhole they cover all slots.
                _dummies = []
                for i in range(XP_BUFS):
                    zt = xp.tile([128, 32 * XW], F32R, tag="X", name=f"xz{i}")
                    pads = bass.AP(tensor=zt.tensor, offset=zt.offset,
                                   ap=[zt.ap[0], [XW, 32], [XW - 1, 2]])
                    nc.vector.memset(pads.bitcast(F32), 0.0)
                    _dummies.append(zt)
                _pad_scratch = small.tile([128, 1], F32)
                for zt in _dummies:
                    nc.scalar.copy(out=_pad_scratch,
                                   in_=zt.bitcast(F32)[:, 0:1])

                # which (L, s, g) depthwise instances run on DVE (tap
                # products) + DMA engines (tap accumulation) instead of PE
                def offloaded(L, s, g):
                    # g2 -> DVE fused chain; the rest stay on PE. L2 stays
                    # on PE too: its chains would head-of-line-block L1's
                    # remaining stats on the in-order Vector queue.
                    return L >= 3 and g == 2

                def wsm(g, t):
                    c = WDWS_OFF + g * 9 + t
                    return w32[:, c:c + 1]

                OffD = {}

                def emit_offload(Ln, g, s, xt):
                    # full depthwise instance on DVE: tap (1,1) initializes,
                    # the rest accumulate in place with fused (x*w)+acc
                    dD = dp.tile([128, 1024], BF16, tag="D",
                                 name=f"OD{Ln}_{s}_{g}")
                    dDr = dD.rearrange("p (y x) -> p y x", y=32, x=32)
                    X34 = xt.bitcast(F32).rearrange("p (y x) -> p y x",
                                                    y=32, x=XW)
                    nc.vector.tensor_scalar_mul(
                        out=dD, in0=X34[:, 0:32, 1:33], scalar1=wsm(g, 0))
                    for ti in range(1, 9):
                        dy, dx = TAPS[ti]
                        oy0, oy1 = max(0, 1 - dy), min(32, 33 - dy)
                        acc = dDr[:, oy0:oy1, :]
                        nc.vector.scalar_tensor_tensor(
                            out=acc,
                            in0=X34[:, oy0 + dy - 1:oy1 + dy - 1,
                                    dx:dx + 32],
                            scalar=wsm(g, ti), in1=acc,
                            op0=mybir.AluOpType.mult,
                            op1=mybir.AluOpType.add)
                    OffD[(Ln, g, s)] = dD

                def emit_offload_ap(Ln, g, s, xt):
                    # taps on ScalarE (copy with per-channel scale), in-place
                    # accumulation on the Pool engine
                    dD = dp.tile([128, 1024], BF16, tag="D",
                                 name=f"OA{Ln}_{s}_{g}")
                    dDr = dD.rearrange("p (y x) -> p y x", y=32, x=32)
                    X36 = xt.rearrange("p (y x) -> p y x", y=32, x=XW)
                    nc.scalar.activation(out=dD, in_=X36[:, 0:32, 2:34],
                                         func=AF.Copy, scale=wsm(g, 0))
                    for ti in range(1, 9):
                        dy, dx = TAPS[ti]
                        oy0, oy1 = max(0, 1 - dy), min(32, 33 - dy)
                        n = oy1 - oy0
                        tb = tp.tile([128, 1024], BF16, tag="T",
                                     name=f"ta{Ln}_{s}_{g}_{ti}")
                        tbr = tb.rearrange("p (y x) -> p y x", y=32, x=32)
                        nc.scalar.activation(
                            out=tbr[:, 0:n, :],
                            in_=X36[:, oy0 + dy - 1:oy1 + dy - 1,
                                    dx + 1:dx + 33],
                            func=AF.Copy, scale=wsm(g, ti))
                        acc = dDr[:, oy0:oy1, :]
                        nc.gpsimd.tensor_tensor(out=acc, in0=tbr[:, 0:n, :],
                                                in1=acc,
                                                op=mybir.AluOpType.add)
                    OffD[(Ln, g, s)] = dD

                Xcur = {}
                for s in range(BPC):
                    xt = new_x_tile(f"X1_0_{s}")
                    xtr = xt.rearrange("p (y x) -> p y x", y=32, x=XW)
                    h3r = h3.rearrange("p (s y x) -> p s y x", s=4, y=32, x=32)
                    nc.scalar.activation(out=xtr[:, :, 1:33],
                                         in_=h3r[:, s, :, :],
                                         func=AF.Relu,
                                         scale=s2t2[:, 0:1], bias=s2t2[:, 1:2])
                    Xcur[(0, s)] = xt

                pooled_in = small.tile([128, 2, 4, 4], F32)
                tadd = small.tile([128, 2, 4], F32)
                tadd2 = small.tile([128, 2, 4], F32)
                pooled = small.tile([128, 2, 4], F32R)

                for L in range(1, LAYERS + 1):
                    gs_in = sorted({g for (g, _s) in Xcur})
                    mgs = [2, 3] if L == LAYERS else [0, 1, 2, 3]
                    Xnext = {}
                    Dcur = {}
                    for s in range(BPC):
                        # depthwise conv for each live group
                        for g in gs_in:
                            if (L, g, s) in OffD:
                                Dcur[g] = OffD.pop((L, g, s))
                                continue
                            Xr = Xcur[(g, s)].rearrange("p (y x) -> p y x",
                                                        y=32, x=XW)
                            dD = dp.tile([128, 1024], BF16, tag="D",
                                         name=f"D{L}_{s}_{g}")
                            for h in range(2):
                                pdw = psA.tile([128, 512], F32, tag="a",
                                               name=f"pdw{L}_{s}_{g}_{h}")
                                pdwr = pdw.rearrange("p (y x) -> p y x",
                                                     y=16, x=32)
                                for ti, (dy, dx) in enumerate(TAPS):
                                    oy0, oy1 = _clip(h * 16, h * 16 + 16,
                                                     max(0, 1 - dy), 33 - dy)
                                    if oy0 >= oy1:
                                        continue
                                    rhs = Xr[:, oy0 + dy - 1:oy1 + dy - 1,
                                             dx:dx + 32]
                                    outp = pdwr[:, oy0 - h * 16:oy1 - h * 16, :]
                                    t = TAPS.index((dy, dx))
                                    nc.tensor.matmul(
                                        outp, wdw_blk(g, t), rhs,
                                        start=(ti == 0),
                                        stop=(ti == len(TAPS) - 1))
                                nc.scalar.copy(
                                    out=dD[:, h * 512:h * 512 + 512], in_=pdw)
                            Dcur[g] = dD
                        # channel mix + instnorm per output group; offloaded
                        # D groups are accumulated last (they arrive latest)
                        kg_ord = ([g for g in gs_in if not offloaded(L, s, g)]
                                  + [g for g in gs_in if offloaded(L, s, g)])
                        for mg in mgs:
                            pm = psB.tile([128, 1024], F32, tag="b",
                                          name=f"pm{L}_{s}_{mg}")
                            for h in range(2):
                                for ki, kg in enumerate(kg_ord):
                                    nc.tensor.matmul(
                                        pm[:, h * 512:h * 512 + 512],
                                        wmix_blk(kg, mg),
                                        Dcur[kg][:, h * 512:h * 512 + 512],
                                        start=(ki == 0),
                                        stop=(ki == len(kg_ord) - 1))
                            st = stp.tile([128, 2, 6], F32, tag="st")
                            nc.vector.bn_stats(out=st[:, 0, :],
                                               in_=pm[:, 0:512])
                            nc.vector.bn_stats(out=st[:, 1, :],
                                               in_=pm[:, 512:1024])
                            mv = stp.tile([128, 2], F32, tag="mv")
                            nc.vector.bn_aggr(out=mv, in_=st)
                            sc = stp.tile([128, 1], F32, tag="sc")
                            tt = stp.tile([128, 1], F32, tag="tt")
                            nc.scalar.activation(out=sc, in_=mv[:, 1:2],
                                                 func=AF.Sqrt, bias=eps_t)
                            nc.vector.reciprocal(out=sc, in_=sc)
                            nc.vector.tensor_scalar_mul(
                                out=sc, in0=sc,
                                scalar1=w32[:, NGB_OFF + 2 * mg:
                                            NGB_OFF + 2 * mg + 1])
                            nc.vector.tensor_mul(out=tt, in0=mv[:, 0:1], in1=sc)
                            nc.vector.tensor_scalar(
                                out=tt, in0=tt, scalar1=-1.0,
                                scalar2=w32[:, NGB_OFF + 2 * mg + 1:
                                            NGB_OFF + 2 * mg + 2],
                                op0=mybir.AluOpType.mult,
                                op1=mybir.AluOpType.add)
                            if L < LAYERS:
                                xt = new_x_tile(f"X{L + 1}_{mg}_{s}")
                                Xnext[(mg, s)] = xt
                                xtr = xt.rearrange("p (y x) -> p y x",
                                                   y=32, x=XW)
                                pmr2 = pm.rearrange("p (y x) -> p y x",
                                                    y=32, x=32)
                                nc.scalar.activation(
                                    out=xtr[:, :, 1:33], in_=pmr2,
                                    func=AF.Relu, scale=sc, bias=tt)
                                # start the offloaded depthwise for the next
                                # layer as soon as its input exists
                                if offloaded(L + 1, s, mg):
                                    emit_offload(L + 1, mg, s, xt)
                            else:
                                pmr = pm.rearrange("p (h y x) -> p h y x",
                                                   h=2, y=16, x=32)
                                nc.scalar.activation(
                                    out=pooled_in[:, mg - 2, s, :],
                                    in_=pmr[:, 0, HALF - 1:HALF + 1,
                                            HALF - 1:HALF + 1],
                                    func=AF.Identity, scale=sc, bias=tt)
                                if mg == 3:
                                    # fold this sample's 2x2 mean right away
                                    # on the idle Pool engine (DVE's queue is
                                    # deep at the end of L8)
                                    nc.gpsimd.tensor_add(
                                        out=tadd[:, :, s],
                                        in0=pooled_in[:, :, s, 0],
                                        in1=pooled_in[:, :, s, 1])
                                    nc.gpsimd.tensor_add(
                                        out=tadd2[:, :, s],
                                        in0=pooled_in[:, :, s, 2],
                                        in1=pooled_in[:, :, s, 3])
                                    nc.gpsimd.tensor_add(
                                        out=pooled[:, :, s],
                                        in0=tadd[:, :, s],
                                        in1=tadd2[:, :, s])
                    Xcur = Xnext

                # ---------------- readout ----------------
                y_sb = small.tile([128, 4, 8], F32)
                for mo in range(8):
                    mlen = 128 if mo < 7 else OUT - 7 * 128
                    pf = psA.tile([128, 512], F32, tag="a", name=f"pf{mo}")
                    for kgi in range(2):
                        nc.tensor.matmul(
                            pf[0:mlen, 0:4],
                            w_sb[:, FCW_OFF + kgi * 1000 + mo * 128:
                                 FCW_OFF + kgi * 1000 + mo * 128 + mlen],
                            pooled[:, kgi, :],
                            start=(kgi == 0), stop=(kgi == 1))
                    nc.scalar.activation(
                        out=y_sb[0:mlen, :, mo], in_=pf[0:mlen, 0:4],
                        func=AF.Identity,
                        bias=w32[0:mlen, FCB_OFF + mo:FCB_OFF + mo + 1],
                        scale=1.0)
                    # ship this mo-block for all samples right away so the
                    # output DMAs pipeline behind the remaining fc matmuls
                    dsty = bass.AP(tensor=y4.tensor, offset=128 * mo,
                                   ap=[[1, mlen], [OUT, 4]])
                    nc.sync.dma_start(out=dsty, in_=y_sb[0:mlen, :, mo])

    nc.finalize()
    return nc


def _bn_scale_bias(nc, out_st, mom, w32, gb_off, eps_t, pool):
    """mom = [mu, E[x^2]] per partition.
    out_st[:, 0] = gamma*rsqrt(var+eps); out_st[:, 1] = beta - mu*scale."""
    var = pool.tile([128, 1], F32, name=f"var{gb_off}")
    mus = pool.tile([128, 1], F32, name=f"mus{gb_off}")
    nc.vector.tensor_mul(out=var, in0=mom[:, 0:1], in1=mom[:, 0:1])
    nc.vector.tensor_sub(out=var, in0=mom[:, 1:2], in1=var)
    nc.scalar.activation(out=var, in_=var, func=AF.Sqrt, bias=eps_t)
    nc.vector.reciprocal(out=var, in_=var)
    nc.vector.tensor_scalar_mul(out=out_st[:, 0:1], in0=var,
                                scalar1=w32[:, gb_off:gb_off + 1])
    nc.vector.tensor_mul(out=mus, in0=mom[:, 0:1], in1=out_st[:, 0:1])
    nc.vector.tensor_scalar(out=out_st[:, 1:2], in0=mus, scalar1=-1.0,
                            scalar2=w32[:, gb_off + 1:gb_off + 2],
                            op0=mybir.AluOpType.mult,
                            op1=mybir.AluOpType.add)


def _pack_weights(ds_w1, ds_w2, ds_w3, conv_w, graph_w, fc_w, fc_b,
                  bn1_g, bn1_b, bn2_g, bn2_b, norm_g, norm_b):
    import ml_dtypes
    wts_u16 = np.zeros((128, WCOLS * 2), np.uint16)
    wts = wts_u16.view(np.float32)      # f32 view of the same buffer
    # pruned graph weight
    k = int((1.0 - PRUNE) * DIM * DIM)
    a = np.abs(graph_w).ravel()
    thresh = np.partition(a, -k)[-k]
    w_eff = np.where(np.abs(graph_w) >= thresh, graph_w, 0.0).astype(np.float32)
    # conv1 taps, paired block-diag:
    # rows 64*q + 27*a + 3*dy + c, cols 64*a + o = w1[o, c, dy, dx]
    for dx in range(3):
        dc = di_col(dx)
        blk = np.zeros((128, 128), np.float32)
        for qq in range(2):
            for aa in range(2):
                for dy in range(3):
                    for c in range(3):
                        blk[64 * qq + 27 * aa + 3 * dy + c,
                            64 * aa:64 * aa + 64] = ds_w1[:, c, dy, dx]
        wts[:, W1X_OFF + dc * 128:W1X_OFF + (dc + 1) * 128] = blk
    # conv2 diag-dup taps
    for t, (dy, dx) in enumerate(TAPS):
        blk = np.zeros((128, 128), np.float32)
        d = ds_w2[:, 0, dy, dx]
        for aa in range(2):
            idx = np.arange(64)
            blk[64 * aa + idx, 64 * aa + idx] = d
        wts[:, W2D_OFF + t * 128:W2D_OFF + (t + 1) * 128] = blk
    # conv3: [64a + c, o] = w3[o, c]
    w3 = ds_w3[:, :, 0, 0]  # [128, 64]
    wts[0:64, W3_OFF:W3_OFF + 128] = w3.T
    wts[64:128, W3_OFF:W3_OFF + 128] = w3.T
    # main dw diag taps (f32r)
    for g in range(4):
        for t, (dy, dx) in enumerate(TAPS):
            blk = np.zeros((128, 128), np.float32)
            idx = np.arange(128)
            blk[idx, idx] = conv_w[g * 128:(g + 1) * 128, 0, dy, dx]
            off = WDW_OFF + (g * 9 + t) * 128
            wts[:, off:off + 128] = blk
            # compact per-channel tap column for the DVE-offloaded instances
            wts[:, WDWS_OFF + g * 9 + t] = conv_w[g * 128:(g + 1) * 128,
                                                  0, dy, dx]
    # mix (fp16): [p, kg*512 + mg*128 + j] = w_eff[mg*128 + j, kg*128 + p]
    weT = w_eff.T  # [in, out]
    for kg in range(4):
        hb = np.asarray(weT[kg * 128:(kg + 1) * 128, :],
                        dtype=np.float16).view(np.uint16)
        off = 2 * WHX_OFF + kg * 512
        wts_u16[:, off:off + 512] = hb
    # fc: [p, kg*1000 + m] = 0.25 * fc_w[m, kg*128 + p]
    for kg in range(2):
        wts[:, FCW_OFF + kg * 1000:FCW_OFF + (kg + 1) * 1000] = \
            0.25 * fc_w[:, kg * 128:(kg + 1) * 128].T
    # fc bias [p, mo]
    fcb = np.zeros((128, 8), np.float32)
    fb = np.zeros(1024, np.float32)
    fb[:OUT] = fc_b
    fcb[:, :] = fb.reshape(8, 128).T
    wts[:, FCB_OFF:FCB_OFF + 8] = fcb
    # bn gammas/betas
    wts[0:64, BN1_OFF] = bn1_g
    wts[64:128, BN1_OFF] = bn1_g
    wts[0:64, BN1_OFF + 1] = bn1_b
    wts[64:128, BN1_OFF + 1] = bn1_b
    wts[:, BN2_OFF] = bn2_g
    wts[:, BN2_OFF + 1] = bn2_b
    for g in range(4):
        wts[:, NGB_OFF + 2 * g] = norm_g[g * 128:(g + 1) * 128]
        wts[:, NGB_OFF + 2 * g + 1] = norm_b[g * 128:(g + 1) * 128]
    return wts


_nc_cache = None
last_results = None


def kernel(**inputs):
    global _nc_cache, last_results
    inputs = {k: np.asarray(v, np.float32) for k, v in inputs.items()}
    wts = _pack_weights(
        inputs["ds_w1"], inputs["ds_w2"], inputs["ds_w3"], inputs["conv_w"],
        inputs["graph_w"], inputs["fc_w"], inputs["fc_b"],
        inputs["bn1_g"], inputs["bn1_b"], inputs["bn2_g"], inputs["bn2_b"],
        inputs["norm_g"], inputs["norm_b"])
    x = inputs["x"]
    if _nc_cache is None:
        _nc_cache = build_nc()
    nc = _nc_cache
    in_maps = [{"x4": np.ascontiguousarray(x[c * BPC:(c + 1) * BPC]),
                "wts": wts} for c in range(N_CORES)]
    res = run_bass_kernel_spmd(nc, in_maps, core_ids=list(range(N_CORES)))
    last_results = res
    return np.concatenate([res.results[c]["y4"] for c in range(N_CORES)], axis=0)


# revision 59
# speedup vs baseline: 1.0000x; 1.0000x over previous
"""Trainium2 Bass kernel for nn_DiscreteTimeNeuralGraph.

Strategy (8 NeuronCores, batch-parallel):
  - Shard the batch of 32 across 8 cores (4 samples each). All weights
    replicated; weight DMA split in three (downsample chunk first) so the
    downsample path starts before the bulky main-loop weights land.
  - Downsample path on-device; BatchNorm batch stats via per-core partial
    moments + one small AllGather each (collective AllReduce has a ~2x
    higher fixed cost in practice) followed by a local free-dim reduce.
  - All zero-padding of activation borders is done with strided memsets on
    the Vector engine, never with DMAs (which would serialize on a queue).
  - 8 graph layers: depthwise 3x3 conv as 9 rect-clipped diagonal matmuls
    on PE accumulating in PSUM; channel mix (pruned 512x512 weight, dense)
    as blocked fp16 matmuls; instance-norm stats on VectorE (bn_stats on
    PSUM); instnorm+ReLU fused into one ScalarE activation reading PSUM
    and writing the next layer's activations (f32r).
  - Engine balance: for layers 3..8, the depthwise conv of channel group 2
    runs entirely on the Vector engine (per-channel scalar_tensor_tensor
    accumulation chains, emitted one layer ahead right after the producing
    activation) while PE keeps groups 0/1/3 and the channel mix. Starting
    at L3 (not L2) avoids stalling on chains that would have to hide under
    the short first layer.
  - Precision: activations X and PE depthwise path in f32r; the depthwise
    outputs D and the mix weights are fp16 (10-bit mantissa), which
    halves their SBUF footprint; PSUM accumulation stays f32.
  - Readout: center 2x2 mean (folded into fc weights, accumulated
    per-sample as L8 finishes) + fc matmul.

Top-k threshold for the pruned graph weight is computed on host
(np.partition) -- it is weight preprocessing of a replicated input.
"""
import numpy as np

import concourse.bass as bass
import concourse.tile as tile
from concourse import bacc, mybir
from concourse.bass_utils import run_bass_kernel_spmd

F32 = mybir.dt.float32
F32R = mybir.dt.float32r
AF = mybir.ActivationFunctionType

N_CORES = 8
B = 32
BPC = B // N_CORES          # 4 samples per core
DIM = 512
DS = 128
FEAT = 256
LAYERS = 8
IMG = 128
OUT = 1000
EPS = 1e-5
HALF = IMG // 4 // 2 - 1    # 15
PRUNE = 0.9

# mega-weight column layout (f32r, [128, WCOLS]); ds-phase block first so
# the first DMA chunk unblocks conv1 quickly.
W1X_OFF = 0                  # 3 dx-taps x [128,128] for conv1
W2D_OFF = W1X_OFF + 3 * 128  # 9 taps x [128,128] diag-dup for conv2
W3_OFF = W2D_OFF + 9 * 128   # [128,128] conv3 (w3 stacked twice on K)
BN1_OFF = W3_OFF + 128       # [128, 2] bn1 gamma/beta (dup across halves)
BN2_OFF = BN1_OFF + 2        # [128, 2]
DS_COLS = BN2_OFF + 2        # end of ds-phase chunk
WDW_OFF = DS_COLS              # 4 groups x 9 taps x [128,128] diag, f32r
FCW_OFF = WDW_OFF + 36 * 128   # 2 kg x [128, 1000] fc lhsT (x0.25 pooled)
FCB_OFF = FCW_OFF + 2 * 1000   # [128, 8] fc bias chunks
NGB_OFF = FCB_OFF + 8          # [128, 8] instnorm gamma/beta per group
WDWS_OFF = NGB_OFF + 8         # [128, 36] compact dw taps (per-channel) for
                               # the DVE-offloaded depthwise instances
WSB_COLS = WDWS_OFF + 36       # end of the f32r weight tile
# fp16 mix weights (packed in pairs into the f32 dram buffer):
# 4 kg x [128,512] blocks; D is fp16 so the mix runs fp16 x fp16
WHX_OFF = WSB_COLS             # in f32-column units of the dram buffer
WHX_HCOLS = 4 * 512
WCOLS = WHX_OFF + WHX_HCOLS // 2

XP_BUFS = 20
XW = 34                        # X row width: [z | 32 interior | z] (f32r)
BF16 = mybir.dt.float16  # fp16: 10-bit mantissa, same PE/DVE speed as bf16

# tap order: full-coverage tap first (start=True zeroes the psum region)
TAPS = [(1, 1), (0, 0), (0, 1), (0, 2), (1, 0), (1, 2), (2, 0), (2, 1), (2, 2)]


def _clip(lo, hi, lo2, hi2):
    return max(lo, lo2), min(hi, hi2)


def di_col(dx):
    # column index of conv1 tap dx within w1x block (emission order 1,0,2)
    return {1: 0, 0: 1, 2: 2}[dx]


def build_nc():
    nc = bacc.Bacc(num_devices=N_CORES)
    x4 = nc.dram_tensor("x4", [BPC, 3, IMG, IMG], F32R, kind="ExternalInput").ap()
    wts = nc.dram_tensor("wts", [128, WCOLS], F32R, kind="ExternalInput").ap()
    y4 = nc.dram_tensor("y4", [BPC, OUT], F32, kind="ExternalOutput").ap()

    with tile.TileContext(nc) as tc:
        with (
            tc.tile_pool(name="wp", bufs=1) as wp,
            tc.tile_pool(name="small", bufs=1) as small,
            tc.tile_pool(name="psA", bufs=2, space="PSUM") as psA,
            tc.tile_pool(name="psB", bufs=3, space="PSUM") as psB,
            tc.tile_pool(name="dram", bufs=1, space="DRAM") as dram,
        ):
            w_sb = wp.tile([128, WSB_COLS], F32R)
            w32 = w_sb.bitcast(F32)
            whx_sb = wp.tile([128, WHX_HCOLS], BF16)

            def wcols(off, n):
                return w_sb[:, off:off + n]

            def wdw_blk(g, t):
                return wcols(WDW_OFF + (g * 9 + t) * 128, 128)

            def wmix_blk(kg, mg):
                c = kg * 512 + mg * 128
                return whx_sb[:, c:c + 128]

            eps_t = small.tile([128, 1], F32)
            nc.vector.memset(eps_t, EPS)

            # ---------------- downsample ----------------
            with tc.tile_pool(name="ds1", bufs=1) as ds1:
                # im2col9: partition p = 32*s + 3*dy + c ; free = (oy 64, ix' 130)
                # ix' = ix + 1 (x padded by 1 on both sides)
                im9 = ds1.tile([128, 64 * 130], F32R)
                im9r = im9.rearrange("p (y x) -> p y x", y=64, x=130)
                # zero the x pads (cols 0 and 129) with strided memsets
                for xc in (0, 129):
                    im9_pads = bass.AP(tensor=im9.tensor,
                                       offset=im9.offset + xc,
                                       ap=[im9.ap[0], [130, 64]])
                    nc.vector.memset(im9_pads.bitcast(F32), 0.0)
                # row oy=0 zeroed (only the dy=0 partitions keep it)
                nc.vector.memset(im9r[:, 0:1, :].bitcast(F32), 0.0)
                # x rows: iy = 2*oy + dy - 1
                # partition base: sample s -> 64*(s%2) + 27*(s//2)
                x4r = x4.rearrange("s c (y2 two) x -> s c y2 two x", two=2)
                for s in range(BPC):
                    for dy in range(3):
                        p0 = 64 * (s % 2) + 27 * (s // 2) + 3 * dy
                        if dy == 0:
                            # oy in [1,64): iy = 2*(oy-1)+1
                            nc.sync.dma_start(
                                out=im9r[p0:p0 + 3, 1:64, 1:129],
                                in_=x4r[s, :, 0:63, 1, :])
                        elif dy == 1:
                            nc.sync.dma_start(
                                out=im9r[p0:p0 + 3, :, 1:129],
                                in_=x4r[s, :, :, 0, :])
                        else:
                            nc.sync.dma_start(
                                out=im9r[p0:p0 + 3, :, 1:129],
                                in_=x4r[s, :, :, 1, :])
                # weight DMAs after the input: ds-phase chunk unblocks conv1;
                # the bulky main-loop chunks overlap conv1/BN syncs. The
                # bf16-packed dw-tap region must move as bf16 (an f32r-typed
                # DMA rounds mantissas, corrupting packed bf16 pairs).
                nc.sync.dma_start(out=w_sb[:, 0:DS_COLS], in_=wts[:, 0:DS_COLS])
                nc.sync.dma_start(out=w_sb[:, DS_COLS:WSB_COLS],
                                  in_=wts[:, DS_COLS:WSB_COLS])
                nc.sync.dma_start(out=whx_sb,
                                  in_=wts[:, WHX_OFF:WCOLS].bitcast(BF16))

                # conv1: out h1 [128 = 64*(s//2)+ch, (s%2, oy 64, ox 64)]
                h1 = ds1.tile([128, 8192], F32)
                h1r = h1.rearrange("p (sh y x) -> p sh y x", sh=2, y=64, x=64)
                # im2col x-read: ix' = 2*ox + dx (x2 = ox + dx//2, tx = dx%2)
                # paired matmul: K=54 block-diag covers samples (q, q+2):
                # out partitions 0-63 <- sample q, 64-127 <- sample q+2.
                im9x = im9.rearrange("p (y x2 two) -> p y x2 two", x2=65, two=2)
                for q in range(2):
                    for yb in range(4):           # 16-oy blocks
                        for h in range(2):
                            pc1 = psA.tile([128, 512], F32, tag="a",
                                           name="pc1")
                            pc1r = pc1.rearrange("p (y x) -> p y x", y=8, x=64)
                            oy0 = yb * 16 + h * 8
                            for di, dx in enumerate([1, 0, 2]):
                                rhs = im9x[64 * q:64 * q + 54, oy0:oy0 + 8,
                                           dx // 2:dx // 2 + 64, dx % 2]
                                lhsT = w_sb[64 * q:64 * q + 54,
                                            W1X_OFF + di_col(dx) * 128:
                                            W1X_OFF + di_col(dx) * 128 + 128]
                                nc.tensor.matmul(pc1r, lhsT, rhs,
                                                 start=(di == 0), stop=(di == 2),
                                                 tile_position=(64 * q, 0))
                            if (q + yb + h) % 2 == 0:
                                nc.scalar.copy(
                                    out=h1r[:, q, oy0:oy0 + 8, :], in_=pc1)
                            else:
                                nc.vector.tensor_copy(
                                    out=h1r[:, q, oy0:oy0 + 8, :], in_=pc1)

                # BN1 partial stats
                st1 = small.tile([128, 16, 6], F32)
                for i in range(16):
                    nc.vector.bn_stats(out=st1[:, i, :],
                                       in_=h1[:, i * 512:(i + 1) * 512])
                mv1 = small.tile([128, 2], F32)
                nc.vector.bn_aggr(out=mv1, in_=st1)
                # prescaled moments: global mean = sum of the 16 (core, half)
                # local means / 16, ditto E[x^2]
                sums1 = small.tile([128, 2], F32)
                tmp1 = small.tile([128, 1], F32)
                nc.vector.tensor_scalar_mul(out=sums1[:, 0:1], in0=mv1[:, 0:1],
                                            scalar1=1.0 / 16.0)
                nc.vector.tensor_mul(out=tmp1, in0=mv1[:, 0:1], in1=mv1[:, 0:1])
                nc.vector.tensor_add(out=tmp1, in0=tmp1, in1=mv1[:, 1:2])
                nc.vector.tensor_scalar_mul(out=sums1[:, 1:2], in0=tmp1,
                                            scalar1=1.0 / 16.0)
                bn1_in = dram.tile([128, 2], F32)
                bn1_g8 = dram.tile([8, 128, 2], F32)
                nc.gpsimd.dma_start(out=bn1_in, in_=sums1)
                nc.gpsimd.collective_compute(
                    "AllGather", mybir.AluOpType.bypass,
                    replica_groups=[list(range(N_CORES))],
                    ins=[bn1_in.opt()], outs=[bn1_g8.opt()])
                # both sample-halves of channel c land on partitions c and
                # c+64 via two parallel DMAs reading the same [ch, s, g, half]
                # pattern; one reduce over the 16 (g, half) entries.
                gath1 = small.tile([128, 2, 16], F32)
                src1 = bass.AP(tensor=bn1_g8.tensor, offset=bn1_g8.offset,
                               ap=[[2, 64], [1, 2], [256, 8], [128, 2]])
                nc.gpsimd.dma_start(out=gath1[0:64], in_=src1)
                nc.sync.dma_start(out=gath1[64:128], in_=src1)
                red1 = small.tile([128, 2], F32)
                nc.vector.tensor_reduce(out=red1, in_=gath1,
                                        axis=mybir.AxisListType.X,
                                        op=mybir.AluOpType.add)
                s1t1 = small.tile([128, 2], F32)
                _bn_scale_bias(nc, s1t1, red1, w32, BN1_OFF, eps_t, small)

                # apply BN1 + relu -> h1n (f32r), x padded to 66 (ix' = ix+1)
                h1n = ds1.tile([128, 2 * 64 * 66], F32R)
                h1nr3 = h1n.rearrange("p (sh y x) -> p sh y x",
                                      sh=2, y=64, x=66)
                for sh in range(2):
                    h1n_pads = bass.AP(tensor=h1n.tensor,
                                       offset=h1n.offset + 4224 * sh,
                                       ap=[h1n.ap[0], [66, 64], [65, 2]])
                    nc.vector.memset(h1n_pads.bitcast(F32), 0.0)
                # chunked apply for pipelining into conv2
                for sh in range(2):
                    for yh in range(2):
                        nc.scalar.activation(
                            out=h1nr3[:, sh, yh * 32:yh * 32 + 32, 1:65],
                            in_=h1r[:, sh, yh * 32:yh * 32 + 32, :],
                            func=AF.Relu,
                            scale=s1t1[:, 0:1], bias=s1t1[:, 1:2])

                # conv2: depthwise 3x3 stride 2 -> d2 [128, (sh, 32, 32)]
                # row iy = 2*oy + dy - 1 (unpadded), col ix' = 2*ox + dx (padded)
                h1nr = h1n.rearrange(
                    "p (sh y2 ty x2 tx) -> p sh y2 ty x2 tx",
                    sh=2, y2=32, ty=2, x2=33, tx=2)
                d2 = ds1.tile([128, 2048], F32R)
                for sh in range(2):
                    pd2 = psB.tile([128, 1024], F32, tag="b", name="pd2")
                    pd2r = pd2.rearrange("p (h y x) -> p h y x", h=2, y=16, x=32)
                    for h in range(2):
                        for ti, (dy, dx) in enumerate(TAPS):
                            oy0, oy1 = _clip(h * 16, h * 16 + 16,
                                             1 if dy == 0 else 0, 32)
                            if dy == 1:
                                ys, par = oy0, 0
                            elif dy == 0:
                                ys, par = oy0 - 1, 1
                            else:
                                ys, par = oy0, 1
                            rhs = h1nr[:, sh, ys:ys + (oy1 - oy0), par,
                                       dx // 2:dx // 2 + 32, dx % 2]
                            outp = pd2r[:, h, oy0 - h * 16:oy1 - h * 16, :]
                            t = TAPS.index((dy, dx))
                            nc.tensor.matmul(
                                outp, wcols(W2D_OFF + t * 128, 128), rhs,
                                start=(ti == 0), stop=(ti == len(TAPS) - 1))
                    nc.scalar.copy(out=d2[:, sh * 1024:(sh + 1) * 1024], in_=pd2)

                # conv3: 1x1, 64 -> 128 ; h3 [128=outc, (s, 1024px)]
                # BN2 partial stats interleave with the conv3 psum copies
                h3 = small.tile([128, 4096], F32)
                st2 = small.tile([128, 8, 6], F32)
                for a in range(2):
                    for nb in range(4):
                        pc3 = psA.tile([128, 512], F32, tag="a",
                                       name=f"pc3_{a}_{nb}")
                        nc.tensor.matmul(
                            pc3,
                            w_sb[64 * a:64 * a + 64, W3_OFF:W3_OFF + 128],
                            d2[64 * a:64 * a + 64, nb * 512:(nb + 1) * 512],
                            start=True, stop=True)
                        s_full = 2 * a + nb // 2
                        ci = s_full * 2 + (nb % 2)
                        dst = h3[:, ci * 512:ci * 512 + 512]
                        nc.scalar.copy(out=dst, in_=pc3)
                        nc.vector.bn_stats(out=st2[:, ci, :], in_=dst)
                mv2 = small.tile([128, 2], F32)
                nc.vector.bn_aggr(out=mv2, in_=st2)
                sums2 = small.tile([128, 2], F32)
                tmp2 = small.tile([128, 1], F32)
                nc.vector.tensor_scalar_mul(out=sums2[:, 0:1], in0=mv2[:, 0:1],
                                            scalar1=1.0 / 8.0)
                nc.vector.tensor_mul(out=tmp2, in0=mv2[:, 0:1], in1=mv2[:, 0:1])
                nc.vector.tensor_add(out=tmp2, in0=tmp2, in1=mv2[:, 1:2])
                nc.vector.tensor_scalar_mul(out=sums2[:, 1:2], in0=tmp2,
                                            scalar1=1.0 / 8.0)
                bn2_in = dram.tile([128, 2], F32)
                bn2_g8 = dram.tile([8, 128, 2], F32)
                nc.gpsimd.dma_start(out=bn2_in, in_=sums2)
                nc.gpsimd.collective_compute(
                    "AllGather", mybir.AluOpType.bypass,
                    replica_groups=[list(range(N_CORES))],
                    ins=[bn2_in.opt()], outs=[bn2_g8.opt()])
                gath2 = small.tile([128, 2, 8], F32)
                src2 = bass.AP(tensor=bn2_g8.tensor, offset=bn2_g8.offset,
                               ap=[[2, 128], [1, 2], [256, 8]])
                nc.gpsimd.dma_start(out=gath2, in_=src2)
                red2 = small.tile([128, 2], F32)
                nc.vector.tensor_reduce(out=red2, in_=gath2,
                                        axis=mybir.AxisListType.X,
                                        op=mybir.AluOpType.add)
                s2t2 = small.tile([128, 2], F32)
                _bn_scale_bias(nc, s2t2, red2, w32, BN2_OFF, eps_t, small)

            # ---------------- main loop ----------------
            with (
                tc.tile_pool(name="xp", bufs=XP_BUFS) as xp,
                tc.tile_pool(name="dp", bufs=16) as dp,
                tc.tile_pool(name="tp", bufs=4) as tp,
                tc.tile_pool(name="stp", bufs=4) as stp,
            ):
                def new_x_tile(name):
                    # pad columns (0, 33) of every xp slot were zeroed once
                    # below; applies only write the interior, so they persist.
                    return xp.tile([128, 32 * XW], F32R, tag="X", name=name)

                # one-time zeroing of the pad columns of all X slots: the
                # dummies are simultaneously live (kept alive by the reads
                # below), so by pigeonhole they cover all slots.
                _dummies = []
                for i in range(XP_BUFS):
                    zt = xp.tile([128, 32 * XW], F32R, tag="X", name=f"xz{i}")
                    pads = bass.AP(tensor=zt.tensor, offset=zt.offset,
                                   ap=[zt.ap[0], [XW, 32], [XW - 1, 2]])
                    nc.vector.memset(pads.bitcast(F32), 0.0)
                    _dummies.append(zt)
                _pad_scratch = small.tile([128, 1], F32)
                for zt in _dummies:
                    nc.scalar.copy(out=_pad_scratch,
                                   in_=zt.bitcast(F32)[:, 0:1])

                # which (L, s, g) depthwise instances run on DVE (tap
                # products) + DMA engines (tap accumulation) instead of PE
                def offloaded(L, s, g):
                    # g2 -> DVE fused chain; the rest stay on PE. L2 stays
                    # on PE too: its chains would head-of-line-block L1's
                    # remaining stats on the in-order Vector queue.
                    return L >= 3 and g == 2

                def wsm(g, t):
                    c = WDWS_OFF + g * 9 + t
                    return w32[:, c:c + 1]

                OffD = {}

                def emit_offload(Ln, g, s, xt):
                    # full depthwise instance on DVE: tap (1,1) initializes,
                    # the rest accumulate in place with fused (x*w)+acc
                    dD = dp.tile([128, 1024], BF16, tag="D",
                                 name=f"OD{Ln}_{s}_{g}")
                    dDr = dD.rearrange("p (y x) -> p y x", y=32, x=32)
                    X34 = xt.bitcast(F32).rearrange("p (y x) -> p y x",
                                                    y=32, x=XW)
                    nc.vector.tensor_scalar_mul(
                        out=dD, in0=X34[:, 0:32, 1:33], scalar1=wsm(g, 0))
                    for ti in range(1, 9):
                        dy, dx = TAPS[ti]
                        oy0, oy1 = max(0, 1 - dy), min(32, 33 - dy)
                        acc = dDr[:, oy0:oy1, :]
                        nc.vector.scalar_tensor_tensor(
                            out=acc,
                            in0=X34[:, oy0 + dy - 1:oy1 + dy - 1,
                                    dx:dx + 32],
                            scalar=wsm(g, ti), in1=acc,
                            op0=mybir.AluOpType.mult,
                            op1=mybir.AluOpType.add)
                    OffD[(Ln, g, s)] = dD

                def emit_offload_ap(Ln, g, s, xt):
                    # taps on ScalarE (copy with per-channel scale), in-place
                    # accumulation on the Pool engine
                    dD = dp.tile([128, 1024], BF16, tag="D",
                                 name=f"OA{Ln}_{s}_{g}")
                    dDr = dD.rearrange("p (y x) -> p y x", y=32, x=32)
                    X36 = xt.rearrange("p (y x) -> p y x", y=32, x=XW)
                    nc.scalar.activation(out=dD, in_=X36[:, 0:32, 2:34],
                                         func=AF.Copy, scale=wsm(g, 0))
                    for ti in range(1, 9):
                        dy, dx = TAPS[ti]
                        oy0, oy1 = max(0, 1 - dy), min(32, 33 - dy)
                        n = oy1 - oy0
                        tb = tp.tile([128, 1024], BF16, tag="T",
                                     name=f"ta{Ln}_{s}_{g}_{ti}")
                        tbr = tb.rearrange("p (y x) -> p y x", y=32, x=32)
                        nc.scalar.activation(
                            out=tbr[:, 0:n, :],
                            in_=X36[:, oy0 + dy - 1:oy1 + dy - 1,
                                    dx + 1:dx + 33],
                            func=AF.Copy, scale=wsm(g, ti))
                        acc = dDr[:, oy0:oy1, :]
                        nc.gpsimd.tensor_tensor(out=acc, in0=tbr[:, 0:n, :],
                                                in1=acc,
                                                op=mybir.AluOpType.add)
                    OffD[(Ln, g, s)] = dD

                Xcur = {}
                for s in range(BPC):
                    xt = new_x_tile(f"X1_0_{s}")
                    xtr = xt.rearrange("p (y x) -> p y x", y=32, x=XW)
                    h3r = h3.rearrange("p (s y x) -> p s y x", s=4, y=32, x=32)
                    nc.scalar.activation(out=xtr[:, :, 1:33],
                                         in_=h3r[:, s, :, :],
                                         func=AF.Relu,
                                         scale=s2t2[:, 0:1], bias=s2t2[:, 1:2])
                    Xcur[(0, s)] = xt

                pooled_in = small.tile([128, 2, 4, 4], F32)
                tadd = small.tile([128, 2, 4], F32)
                tadd2 = small.tile([128, 2, 4], F32)
                pooled = small.tile([128, 2, 4], F32R)

                for L in range(1, LAYERS + 1):
                    gs_in = sorted({g for (g, _s) in Xcur})
                    mgs = [2, 3] if L == LAYERS else [0, 1, 2, 3]
                    Xnext = {}
                    Dcur = {}
                    for s in range(BPC):
                        # depthwise conv for each live group
                        for g in gs_in:
                            if (L, g, s) in OffD:
                                Dcur[g] = OffD.pop((L, g, s))
                                continue
                            Xr = Xcur[(g, s)].rearrange("p (y x) -> p y x",
                                                        y=32, x=XW)
                            dD = dp.tile([128, 1024], BF16, tag="D",
                                         name=f"D{L}_{s}_{g}")
                            for h in range(2):
                                pdw = psA.tile([128, 512], F32, tag="a",
                                               name=f"pdw{L}_{s}_{g}_{h}")
                                pdwr = pdw.rearrange("p (y x) -> p y x",
                                                     y=16, x=32)
                                for ti, (dy, dx) in enumerate(TAPS):
                                    oy0, oy1 = _clip(h * 16, h * 16 + 16,
                                                     max(0, 1 - dy), 33 - dy)
                                    if oy0 >= oy1:
                                        continue
                                    rhs = Xr[:, oy0 + dy - 1:oy1 + dy - 1,
                                             dx:dx + 32]
                                    outp = pdwr[:, oy0 - h * 16:oy1 - h * 16, :]
                                    t = TAPS.index((dy, dx))
                                    nc.tensor.matmul(
                                        outp, wdw_blk(g, t), rhs,
                                        start=(ti == 0),
                                        stop=(ti == len(TAPS) - 1))
                                nc.scalar.copy(
                                    out=dD[:, h * 512:h * 512 + 512], in_=pdw)
                            Dcur[g] = dD
                        # channel mix + instnorm per output group; offloaded
                        # D groups are accumulated last (they arrive latest)
                        kg_ord = ([g for g in gs_in if not offloaded(L, s, g)]
                                  + [g for g in gs_in if offloaded(L, s, g)])
                        for mg in mgs:
                            pm = psB.tile([128, 1024], F32, tag="b",
                                          name=f"pm{L}_{s}_{mg}")
                            for h in range(2):
                                for ki, kg in enumerate(kg_ord):
                                    nc.tensor.matmul(
                                        pm[:, h * 512:h * 512 + 512],
                                        wmix_blk(kg, mg),
                                        Dcur[kg][:, h * 512:h * 512 + 512],
                                        start=(ki == 0),
                                        stop=(ki == len(kg_ord) - 1))
                            st = stp.tile([128, 2, 6], F32, tag="st")
                            nc.vector.bn_stats(out=st[:, 0, :],
                                               in_=pm[:, 0:512])
                            nc.vector.bn_stats(out=st[:, 1, :],
                                               in_=pm[:, 512:1024])
                            mv = stp.tile([128, 2], F32, tag="mv")
                            nc.vector.bn_aggr(out=mv, in_=st)
                            sc = stp.tile([128, 1], F32, tag="sc")
                            tt = stp.tile([128, 1], F32, tag="tt")
                            nc.scalar.activation(out=sc, in_=mv[:, 1:2],
                                                 func=AF.Sqrt, bias=eps_t)
                            nc.vector.reciprocal(out=sc, in_=sc)
                            nc.vector.tensor_scalar_mul(
                                out=sc, in0=sc,
                                scalar1=w32[:, NGB_OFF + 2 * mg:
                                            NGB_OFF + 2 * mg + 1])
                            nc.vector.tensor_mul(out=tt, in0=mv[:, 0:1], in1=sc)
                            nc.vector.tensor_scalar(
                                out=tt, in0=tt, scalar1=-1.0,
                                scalar2=w32[:, NGB_OFF + 2 * mg + 1:
                                            NGB_OFF + 2 * mg + 2],
                                op0=mybir.AluOpType.mult,
                                op1=mybir.AluOpType.add)
                            if L < LAYERS:
                                xt = new_x_tile(f"X{L + 1}_{mg}_{s}")
                                Xnext[(mg, s)] = xt
                                xtr = xt.rearrange("p (y x) -> p y x",
                                                   y=32, x=XW)
                                pmr2 = pm.rearrange("p (y x) -> p y x",
                                                    y=32, x=32)
                                nc.scalar.activation(
                                    out=xtr[:, :, 1:33], in_=pmr2,
                                    func=AF.Relu, scale=sc, bias=tt)
                                # start the offloaded depthwise for the next
                                # layer as soon as its input exists
                                if offloaded(L + 1, s, mg):
                                    emit_offload(L + 1, mg, s, xt)
                            else:
                                pmr = pm.rearrange("p (h y x) -> p h y x",
                                                   h=2, y=16, x=32)
                                nc.scalar.activation(
                                    out=pooled_in[:, mg - 2, s, :],
                                    in_=pmr[:, 0, HALF - 1:HALF + 1,
                                            HALF - 1:HALF + 1],
                                    func=AF.Identity, scale=sc, bias=tt)
                                if mg == 3:
                                    # fold this sample's 2x2 mean right away
                                    nc.vector.tensor_add(
                                        out=tadd[:, :, s],
                                        in0=pooled_in[:, :, s, 0],
                                        in1=pooled_in[:, :, s, 1])
                                    nc.vector.tensor_add(
                                        out=tadd2[:, :, s],
                                        in0=pooled_in[:, :, s, 2],
                                        in1=pooled_in[:, :, s, 3])
                                    nc.vector.tensor_add(
                                        out=pooled[:, :, s],
                                        in0=tadd[:, :, s],
                                        in1=tadd2[:, :, s])
                    Xcur = Xnext

                # ---------------- readout ----------------
                y_sb = small.tile([128, 4, 8], F32)
                for mo in range(8):
                    mlen = 128 if mo < 7 else OUT - 7 * 128
                    pf = psA.tile([128, 512], F32, tag="a", name=f"pf{mo}")
                    for kgi in range(2):
                        nc.tensor.matmul(
                            pf[0:mlen, 0:4],
                            w_sb[:, FCW_OFF + kgi * 1000 + mo * 128:
                                 FCW_OFF + kgi * 1000 + mo * 128 + mlen],
                            pooled[:, kgi, :],
                            start=(kgi == 0), stop=(kgi == 1))
                    nc.scalar.activation(
                        out=y_sb[0:mlen, :, mo], in_=pf[0:mlen, 0:4],
                        func=AF.Identity,
                        bias=w32[0:mlen, FCB_OFF + mo:FCB_OFF + mo + 1],
                        scale=1.0)
                    # ship this mo-block for all samples right away so the
                    # output DMAs pipeline behind the remaining fc matmuls
                    dsty = bass.AP(tensor=y4.tensor, offset=128 * mo,
                                   ap=[[1, mlen], [OUT, 4]])
                    nc.sync.dma_start(out=dsty, in_=y_sb[0:mlen, :, mo])

    nc.finalize()
    return nc


def _bn_scale_bias(nc, out_st, mom, w32, gb_off, eps_t, pool):
    """mom = [mu, E[x^2]] per partition.
    out_st[:, 0] = gamma*rsqrt(var+eps); out_st[:, 1] = beta - mu*scale."""
    var = pool.tile([128, 1], F32, name=f"var{gb_off}")
    mus = pool.tile([128, 1], F32, name=f"mus{gb_off}")
    nc.vector.tensor_mul(out=var, in0=mom[:, 0:1], in1=mom[:, 0:1])
    nc.vector.tensor_sub(out=var, in0=mom[:, 1:2], in1=var)
    nc.scalar.activation(out=var, in_=var, func=AF.Sqrt, bias=eps_t)
    nc.vector.reciprocal(out=var, in_=var)
    nc.vector.tensor_scalar_mul(out=out_st[:, 0:1], in0=var,
                                scalar1=w32[:, gb_off:gb_off + 1])
    nc.vector.tensor_mul(out=mus, in0=mom[:, 0:1], in1=out_st[:, 0:1])
    nc.vector.tensor_scalar(out=out_st[:, 1:2], in0=mus, scalar1=-1.0,
                            scalar2=w32[:, gb_off + 1:gb_off + 2],
                            op0=mybir.AluOpType.mult,
                            op1=mybir.AluOpType.add)


def _pack_weights(ds_w1, ds_w2, ds_w3, conv_w, graph_w, fc_w, fc_b,
                  bn1_g, bn1_b, bn2_g, bn2_b, norm_g, norm_b):
    import ml_dtypes
    wts_u16 = np.zeros((128, WCOLS * 2), np.uint16)
    wts = wts_u16.view(np.float32)      # f32 view of the same buffer
    # pruned graph weight
    k = int((1.0 - PRUNE) * DIM * DIM)
    a = np.abs(graph_w).ravel()
    thresh = np.partition(a, -k)[-k]
    w_eff = np.where(np.abs(graph_w) >= thresh, graph_w, 0.0).astype(np.float32)
    # conv1 taps, paired block-diag:
    # rows 64*q + 27*a + 3*dy + c, cols 64*a + o = w1[o, c, dy, dx]
    for dx in range(3):
        dc = di_col(dx)
        blk = np.zeros((128, 128), np.float32)
        for qq in range(2):
            for aa in range(2):
                for dy in range(3):
                    for c in range(3):
                        blk[64 * qq + 27 * aa + 3 * dy + c,
                            64 * aa:64 * aa + 64] = ds_w1[:, c, dy, dx]
        wts[:, W1X_OFF + dc * 128:W1X_OFF + (dc + 1) * 128] = blk
    # conv2 diag-dup taps
    for t, (dy, dx) in enumerate(TAPS):
        blk = np.zeros((128, 128), np.float32)
        d = ds_w2[:, 0, dy, dx]
        for aa in range(2):
            idx = np.arange(64)
            blk[64 * aa + idx, 64 * aa + idx] = d
        wts[:, W2D_OFF + t * 128:W2D_OFF + (t + 1) * 128] = blk
    # conv3: [64a + c, o] = w3[o, c]
    w3 = ds_w3[:, :, 0, 0]  # [128, 64]
    wts[0:64, W3_OFF:W3_OFF + 128] = w3.T
    wts[64:128, W3_OFF:W3_OFF + 128] = w3.T
    # main dw diag taps (f32r)
    for g in range(4):
        for t, (dy, dx) in enumerate(TAPS):
            blk = np.zeros((128, 128), np.float32)
            idx = np.arange(128)
            blk[idx, idx] = conv_w[g * 128:(g + 1) * 128, 0, dy, dx]
            off = WDW_OFF + (g * 9 + t) * 128
            wts[:, off:off + 128] = blk
            # compact per-channel tap column for the DVE-offloaded instances
            wts[:, WDWS_OFF + g * 9 + t] = conv_w[g * 128:(g + 1) * 128,
                                                  0, dy, dx]
    # mix (fp16): [p, kg*512 + mg*128 + j] = w_eff[mg*128 + j, kg*128 + p]
    weT = w_eff.T  # [in, out]
    for kg in range(4):
        hb = np.asarray(weT[kg * 128:(kg + 1) * 128, :],
                        dtype=np.float16).view(np.uint16)
        off = 2 * WHX_OFF + kg * 512
        wts_u16[:, off:off + 512] = hb
    # fc: [p, kg*1000 + m] = 0.25 * fc_w[m, kg*128 + p]
    for kg in range(2):
        wts[:, FCW_OFF + kg * 1000:FCW_OFF + (kg + 1) * 1000] = \
            0.25 * fc_w[:, kg * 128:(kg + 1) * 128].T
    # fc bias [p, mo]
    fcb = np.zeros((128, 8), np.float32)
    fb = np.zeros(1024, np.float32)
    fb[:OUT] = fc_b
    fcb[:, :] = fb.reshape(8, 128).T
    wts[:, FCB_OFF:FCB_OFF + 8] = fcb
    # bn gammas/betas
    wts[0:64, BN1_OFF] = bn1_g
    wts[64:128, BN1_OFF] = bn1_g
    wts[0:64, BN1_OFF + 1] = bn1_b
    wts[64:128, BN1_OFF + 1] = bn1_b
    wts[:, BN2_OFF] = bn2_g
    wts[:, BN2_OFF + 1] = bn2_b
    for g in range(4):
        wts[:, NGB_OFF + 2 * g] = norm_g[g * 128:(g + 1) * 128]
        wts[:, NGB_OFF + 2 * g + 1] = norm_b[g * 128:(g + 1) * 128]
    return wts


_nc_cache = None
last_results = None


def kernel(**inputs):
    global _nc_cache, last_results
    inputs = {k: np.asarray(v, np.float32) for k, v in inputs.items()}
    wts = _pack_weights(
        inputs["ds_w1"], inputs["ds_w2"], inputs["ds_w3"], inputs["conv_w"],
        inputs["graph_w"], inputs["fc_w"], inputs["fc_b"],
        inputs["bn1_g"], inputs["bn1_b"], inputs["bn2_g"], inputs["bn2_b"],
        inputs["norm_g"], inputs["norm_b"])
    x = inputs["x"]
    if _nc_cache is None:
        _nc_cache = build_nc()
    nc = _nc_cache
    in_maps = [{"x4": np.ascontiguousarray(x[c * BPC:(c + 1) * BPC]),
                "wts": wts} for c in range(N_CORES)]
    res = run_bass_kernel_spmd(nc, in_maps, core_ids=list(range(N_CORES)))
    last_results = res
    return np.concatenate([res.results[c]["y4"] for c in range(N_CORES)], axis=0)
